# revision 18
# baseline (speedup 1.0000x reference)
"""Trainium2 Bass kernel for nn_Block_50706383897045 (dense transformer block).

Data-parallel over batch: B=8 == n_cores, one batch element per core, no
collectives. Per core the full block runs on a [T=1024, C=768] slice.

Key optimizations over the straightforward feature-major design:
- Host-side algebraic folding: both LayerNorms' mean-subtraction is folded
  into column-centered weight matrices ((x-mu)@W == x@center(W)); ln_w/ln_b
  fold into w_qkv/w_c1 and their biases; the K-projection bias is dropped
  (constant-per-query logit shifts cancel in softmax); the V-projection bias
  folds into the out-projection bias (softmax rows sum to 1). On device a
  LayerNorm is just a per-token rstd scale.
- Low-precision matmuls: fp16 for qkv/score/mlp paths, bf16 for the
  exp-weights/V/out-proj path (exp needs bf16 range), fp32 accumulation.
  1 cycle/row on the PE vs ~1.4 for float32r.
- Softmax: scores computed transposed (scoresT[k,q]); denominator from a
  ones-column appended to V; exp is sliced to the causally visible q-range,
  masked region zeroed by cheap DVE memsets + one static bf16 triangular
  [128,128] mask multiply per diagonal tile; reciprocal via the fast DVE
  bit-trick (reciprocal_approx_fast) instead of the 8-cycle/elem divide.
- LN1 stats via DVE bn_stats on the token-major input tiles (overlaps the
  feature-major transposes on PE); rstd rows broadcast across partitions via
  PE ones outer-products; LN2 stats via PE ones-matmuls.
"""
import os
import sys

sys.path.insert(0, "/opt/trn_rl_repo")

import numpy as np
import ml_dtypes

import concourse.bass as bass
import concourse.bacc as bacc
import concourse.mybir as mybir
import concourse.tile as tile
from concourse import bass_utils
from concourse.masks import make_identity

AF = mybir.ActivationFunctionType
ALU = mybir.AluOpType
f32 = mybir.dt.float32
f32r = mybir.dt.float32r
bf16 = mybir.dt.bfloat16
fp16 = mybir.dt.float16

B, T, C, H, D = 8, 1024, 768, 12, 64
F = C // 128      # 6 feature tiles of the residual stream
NT = T // 128     # 8 token tiles
CH = 512          # token chunk
NCH = T // CH     # 2
M3 = 4 * C        # 3072 MLP hidden
MTH = 12          # MLP mid tiles per half
EPS = 1e-5

_NC_CACHE = None


def _chunk(c):
    return slice(c * CH, (c + 1) * CH)


def _build():
    nc = bacc.Bacc("TRN2", target_bir_lowering=False, debug=False,
                   num_devices=8)

    x_d = nc.dram_tensor("x", [T, C], bf16, kind="ExternalInput")
    wqkv_d = nc.dram_tensor("wqkv", [C, 3 * C], fp16, kind="ExternalInput")
    bq_d = nc.dram_tensor("bq", [C], f32, kind="ExternalInput")
    wout_d = nc.dram_tensor("wout", [C, C], bf16, kind="ExternalInput")
    bout_d = nc.dram_tensor("bout", [C], f32, kind="ExternalInput")
    wc1_d = nc.dram_tensor("wc1", [C, M3], fp16, kind="ExternalInput")
    bc1_d = nc.dram_tensor("bc1", [M3], f32, kind="ExternalInput")
    wc2_d = nc.dram_tensor("wc2", [M3, C], fp16, kind="ExternalInput")
    bc2_d = nc.dram_tensor("bc2", [C], f32, kind="ExternalInput")
    y_d = nc.dram_tensor("y", [T, C], f32, kind="ExternalOutput")
    dbg = {}
    if os.environ.get("KDBG"):
        dbg["h"] = nc.dram_tensor("dbg_h", [128, F * T], fp16,
                                  kind="ExternalOutput")
        dbg["kf0"] = nc.dram_tensor("dbg_kf0", [128, T], fp16,
                                    kind="ExternalOutput")
        dbg["qf0"] = nc.dram_tensor("dbg_qf0", [128, T], fp16,
                                    kind="ExternalOutput")
        dbg["v1"] = nc.dram_tensor("dbg_v1", [128, NT * H * 65], bf16,
                                   kind="ExternalOutput")
        dbg["ao"] = nc.dram_tensor("dbg_ao", [128, F * T], bf16,
                                   kind="ExternalOutput")
        dbg["x2"] = nc.dram_tensor("dbg_x2", [128, F * T], bf16,
                                   kind="ExternalOutput")
        dbg["h2"] = nc.dram_tensor("dbg_h2", [128, F * T], fp16,
                                   kind="ExternalOutput")
        dbg["rstd1"] = nc.dram_tensor("dbg_rstd1", [1, T], f32,
                                      kind="ExternalOutput")

    with tile.TileContext(nc) as tc:
        _kernel_body(nc, tc, x_d, wqkv_d, bq_d, wout_d, bout_d,
                     wc1_d, bc1_d, wc2_d, bc2_d, y_d, dbg)
    nc.compile()
    return nc


def _kernel_body(nc, tc, x_d, wqkv_d, bq_d, wout_d, bout_d,
                 wc1_d, bc1_d, wc2_d, bc2_d, y_d, dbg=None):
    dbg = dbg or {}
    with tc.tile_pool(name="persist", bufs=1) as persist:
        ident = persist.tile([128, 128], f32)
        make_identity(nc, ident)
        ones_col = persist.tile([128, 1], bf16)
        nc.vector.memset(ones_col, 1.0)
        ones_row = persist.tile([1, 128], f32r)
        nc.vector.memset(ones_row.bitcast(f32), 1.0)
        eps_t = persist.tile([1, 1], f32)
        nc.vector.memset(eps_t, EPS)
        eps_col = persist.tile([128, 1], f32)
        nc.vector.memset(eps_col, EPS)
        bq_c = persist.tile([128, F], f32)
        nc.sync.dma_start(bq_c, bq_d.ap().rearrange("(o p) -> p o", p=128))
        bout_c = persist.tile([128, F], f32)
        nc.sync.dma_start(bout_c, bout_d.ap().rearrange("(o p) -> p o", p=128))
        bc1_c = persist.tile([128, 24], f32)
        nc.sync.dma_start(bc1_c, bc1_d.ap().rearrange("(o p) -> p o", p=128))
        bc2_c = persist.tile([128, F], f32)
        nc.sync.dma_start(bc2_c, bc2_d.ap().rearrange("(o p) -> p o", p=128))
        # static triangular mask (keep j >= p), bf16
        tri_f = persist.tile([128, 128], f32)
        nc.vector.memset(tri_f, 1.0)
        nc.gpsimd.affine_select(
            out=tri_f, in_=tri_f, compare_op=ALU.is_ge, fill=0.0,
            base=0, pattern=[[1, 128]], channel_multiplier=-1)
        tri = persist.tile([128, 128], bf16)
        nc.vector.tensor_copy(tri, tri_f)

        with (
            tc.tile_pool(name="resid", bufs=1) as resid_pool,
            tc.tile_pool(name="attnout", bufs=1) as ao_pool,
            tc.tile_pool(name="hpool", bufs=1) as h_pool,
            tc.tile_pool(name="rowpool", bufs=1) as row_pool,
        ):
            x_fm = resid_pool.tile([128, F, T], bf16, tag="x_slot",
                                   name="x_fm")
            attn_out = ao_pool.tile([128, F, T], bf16, tag="attn_out",
                                    name="attn_out")
            h_fm = h_pool.tile([128, F, T], fp16, tag="h_slot", name="h_fm")
            rstd1_row = row_pool.tile([1, T], f32r, tag="rstd1",
                                      name="rstd1_row")

            with (
                tc.tile_pool(name="wv", bufs=1) as wv_pool,
                tc.tile_pool(name="wkq", bufs=1) as wkq_pool,
                tc.tile_pool(name="v1pool", bufs=1) as v1_pool,
                tc.tile_pool(name="kqt", bufs=1) as kq_pool,
            ):
                with tc.tile_pool(name="xtm", bufs=1) as xtm_pool:
                    xtm_t = []
                    for t in range(NT):
                        x_tm = xtm_pool.tile([128, C], bf16, tag=f"x_tm{t}",
                                             name=f"x_tm{t}")
                        nc.sync.dma_start(
                            x_tm, x_d.ap()[t * 128:(t + 1) * 128, :])
                        xtm_t.append(x_tm)
                    wv_t, wkq_t = [], []
                    for kt in range(F):
                        wt = wv_pool.tile([128, C], fp16, tag=f"wv{kt}",
                                          name=f"wv{kt}")
                        nc.sync.dma_start(
                            wt, wqkv_d.ap()
                            [kt * 128:(kt + 1) * 128, 2 * C:3 * C])
                        wv_t.append(wt)
                    for kt in range(F):
                        wt = wkq_pool.tile([128, 2 * C], fp16,
                                           tag=f"wkq{kt}", name=f"wkq{kt}")
                        nc.sync.dma_start(
                            wt, wqkv_d.ap()
                            [kt * 128:(kt + 1) * 128, 0:2 * C])
                        wkq_t.append(wt)

                    _phase_a(nc, tc, x_d, xtm_t, x_fm, h_fm, rstd1_row,
                             ident, ones_row, eps_col)
                kf_t, qf_t, v1 = _qkv_attention(
                    nc, tc, v1_pool, kq_pool, wv_t, wkq_t, h_fm,
                    attn_out, bq_c, tri)
                if dbg:
                    nc.sync.dma_start(
                        dbg["h"].ap(),
                        h_fm.rearrange("p f t -> p (f t)"))
                    nc.sync.dma_start(dbg["kf0"].ap(), kf_t[0])
                    nc.sync.dma_start(dbg["qf0"].ap(), qf_t[0])
                    nc.sync.dma_start(
                        dbg["v1"].ap(),
                        v1.rearrange("p t m -> p (t m)"))
                    nc.sync.dma_start(
                        dbg["ao"].ap(),
                        attn_out.rearrange("p f t -> p (f t)"))
                    nc.sync.dma_start(
                        dbg["rstd1"].ap(), rstd1_row.bitcast(f32))

            with (
                tc.tile_pool(name="wc1a", bufs=1) as wc1a_pool,
            ):
                h2_fm, wc1a_t = _outproj_ln2(
                    nc, tc, h_pool, wc1a_pool, row_pool, x_fm, attn_out,
                    wout_d, wc1_d, bout_c, eps_t, ones_col, ones_row)
                if dbg:
                    nc.sync.dma_start(
                        dbg["x2"].ap(),
                        x_fm.rearrange("p f t -> p (f t)"))
                    nc.sync.dma_start(
                        dbg["h2"].ap(),
                        h2_fm.rearrange("p f t -> p (f t)"))
                out_fm = ao_pool.tile([128, F, T], f32, tag="out_slot",
                                      name="out_fm")
                _mlp(nc, tc, wc1a_t, h2_fm, x_fm, out_fm, wc1_d, wc2_d,
                     bc1_c, bc2_c, ident, y_d)


def _phase_a(nc, tc, x_d, xtm_t, x_fm, h_fm, rstd1_row, ident, ones_row,
             eps_col):
    """LN1 stats on token-major tiles (DVE bn_stats) while PE transposes x
    to feature-major; then h = x * rstd (per-token) in fp16."""
    with (
        tc.tile_pool(name="rtps", bufs=2, space="PSUM") as rt_ps,
        tc.tile_pool(name="bcps", bufs=2, space="PSUM") as bc_ps,
        tc.tile_pool(name="bnp", bufs=2) as bn_pool,
        tc.tile_pool(name="rcol", bufs=2) as rc_pool,
    ):
        with nc.named_scope("load_ln1"):
            # feature-major x via DMA crossbar transpose (bf16)
            for f in range(F):
                nc.sync.dma_start_transpose(
                    x_fm[:, f, :], x_d.ap()[:, f * 128:(f + 1) * 128])
            for t in range(NT):
                x_tm = xtm_t[t]
                # bn_stats halves (free max 512) -> aggr -> var col
                bno = bn_pool.tile([128, 2, 6], f32, tag="bno", name="bno")
                nc.vector.bn_stats(bno[:, 0, :], x_tm[:, 0:C // 2])
                nc.vector.bn_stats(bno[:, 1, :], x_tm[:, C // 2:C])
                mv = bn_pool.tile([128, 2], f32, tag="mv", name="mv")
                nc.vector.bn_aggr(mv, bno)
                std = rc_pool.tile([128, 1], f32, tag="std", name="std")
                nc.scalar.activation(std, mv[:, 1:2], AF.Sqrt,
                                     bias=eps_col)
                stdc = rc_pool.tile([128, 1], f32, tag="stdc", name="stdc")
                nc.vector.tensor_copy(stdc, std)
                r0 = rc_pool.tile([128, 1], f32, tag="r0", name="r0")
                nc.vector.reciprocal_approx_fast(out=r0, in_=stdc)
                rstd_col = rc_pool.tile([128, 1], f32, tag="rcol",
                                        name="rstd_col")
                nc.vector.tensor_copy(rstd_col, r0)
                # transpose rstd col -> row segment
                ps_r = rt_ps.tile([1, 128], f32, tag="rt", name="ps_r")
                nc.tensor.transpose(ps_r, rstd_col, ident)
                nc.vector.tensor_copy(
                    rstd1_row[0:1, t * 128:(t + 1) * 128], ps_r)
            for c in range(NCH):
                sl = _chunk(c)
                ps_bc = bc_ps.tile([128, CH], f32, tag="bc", name="ps_bc")
                nc.tensor.matmul(
                    ps_bc, ones_row, rstd1_row[0:1, sl],
                    start=True, stop=True)
                for f in range(F):
                    nc.vector.tensor_mul(
                        h_fm[:, f, sl], x_fm[:, f, sl], ps_bc)


def _qkv_attention(nc, tc, v1_pool, kq_pool, wv_t, wkq_t, h_fm, attn_out,
                   bq_c, tri):
    # V with appended ones column per head (softmax denominator), bf16
    v1 = v1_pool.tile([128, NT, H * 65], bf16, tag="v1", name="v1")
    nc.vector.memset(
        v1.rearrange("p t (h m) -> p t h m", m=65)[:, :, :, 64:65], 1.0)
    with tc.tile_pool(name="vps", bufs=3, space="PSUM") as v_ps:
        with nc.named_scope("qkv_v"):
            for t in range(NT):
                for half in range(2):
                    ps_v = v_ps.tile([128, 384], f32, tag="vps",
                                     name="ps_v")
                    c0 = half * 384
                    for kt in range(F):
                        nc.tensor.matmul(
                            ps_v, h_fm[:, kt, t * 128:(t + 1) * 128],
                            wv_t[kt][:, c0:c0 + 384],
                            start=(kt == 0), stop=(kt == F - 1))
                    dst = (v1[:, t, :].rearrange("p (h m) -> p h m", m=65)
                           [:, half * 6:(half + 1) * 6, 0:64])
                    nc.vector.tensor_copy(
                        dst, ps_v.rearrange("p (h m) -> p h m", m=64))

    with (
        tc.tile_pool(name="ps512", bufs=4, space="PSUM") as ps512,
        tc.tile_pool(name="sps", bufs=2, space="PSUM") as s_ps,
        tc.tile_pool(name="expp", bufs=4) as exp_pool,
        tc.tile_pool(name="attn_sm", bufs=2) as asm_pool,
        tc.tile_pool(name="attn_bcp", bufs=4) as abc_pool,
    ):
        kf_t, qf_t = [], []
        # ---- all KQ projections first ----
        for f in range(F):
            with nc.named_scope(f"kq_{f}"):
                kf = kq_pool.tile([128, T], fp16, tag=f"kf{f}",
                                  name=f"kf{f}")
                qf = kq_pool.tile([128, T], fp16, tag=f"qf{f}",
                                  name=f"qf{f}")
                kf_t.append(kf)
                qf_t.append(qf)
                for c in range(NCH):
                    sl = _chunk(c)
                    ps_k = ps512.tile([128, CH], f32, tag="ps512",
                                      name="kq_psk")
                    for kt in range(F):
                        nc.tensor.matmul(
                            ps_k, wkq_t[kt][:, f * 128:(f + 1) * 128],
                            h_fm[:, kt, sl],
                            start=(kt == 0), stop=(kt == F - 1))
                    nc.vector.tensor_copy(kf[:, sl], ps_k)
                    ps_q = ps512.tile([128, CH], f32, tag="ps512",
                                      name="kq_psq")
                    for kt in range(F):
                        nc.tensor.matmul(
                            ps_q,
                            wkq_t[kt][:, C + f * 128:C + (f + 1) * 128],
                            h_fm[:, kt, sl],
                            start=(kt == 0), stop=(kt == F - 1))
                    nc.vector.tensor_scalar_add(
                        qf[:, sl], ps_q, bq_c[:, f:f + 1])

        # ---- attention: one software-pipelined stream across all heads ----
        # Per (head, kt) unit: score matmuls -> exp/mask -> av matmuls.
        # av for unit i is emitted at unit i+2 so the PE never waits on the
        # scalar engine's exp; the stream crosses head boundaries.
        pend_av = []      # (h_idx, kt, expt, ps_y pair)
        pend_mul = []     # deferred normalize muls
        units = [(2 * f + hl, kt)
                 for f in range(F) for hl in range(2) for kt in range(NT)]
        ps_y_of = {}

        def emit_av(h_idx, kt, ex):
            p0, p1 = ps_y_of[h_idx]
            if kt < 4:
                nc.tensor.matmul(
                    p0[0:65, :], v1[:, kt, h_idx * 65:h_idx * 65 + 65],
                    ex[:, 0:CH], start=(kt == 0), stop=(kt == 3))
            nc.tensor.matmul(
                p1[0:65, :], v1[:, kt, h_idx * 65:h_idx * 65 + 65],
                ex[:, CH:T], start=(kt == 0), stop=(kt == NT - 1))

        def finish_head(h_idx):
            # normalize: fast reciprocal of the ones-row denominator.
            # custom-DVE op deps are invisible to the tile scheduler:
            # sandwich between plain DVE copies (same-engine FIFO).
            f, base = h_idx // 2, (h_idx % 2) * 64
            for half in range(2):
                p_y = ps_y_of[h_idx][half]
                den = asm_pool.tile([1, CH], f32, tag="den", name="den")
                nc.vector.tensor_copy(den, p_y[64:65, :])
                recip0 = asm_pool.tile([1, CH], f32, tag="recip0",
                                       name="recip0")
                nc.vector.reciprocal_approx_fast(out=recip0, in_=den)
                recip = asm_pool.tile([1, CH], f32, tag="recip",
                                      name="recip")
                nc.vector.tensor_copy(recip, recip0)
                bc = abc_pool.tile([64, CH], f32, tag="attn_bc", name="bc")
                nc.gpsimd.partition_broadcast(bc, recip)
                pend_mul.append((p_y, bc, base, f, _chunk(half)))


        for h_idx, kt in units:
            f, base = h_idx // 2, (h_idx % 2) * 64
            kf, qf = kf_t[f], qf_t[f]
            if kt == 0:
                ps_y_of[h_idx] = [
                    ps512.tile([128, CH], f32, tag="ps512",
                               name=f"ps_y{h_idx}_{half}")
                    for half in range(2)]
            ps_s = s_ps.tile([128, T], f32, tag="s1024", name="ps_s")
            ksl = kf[base:base + 64, kt * 128:(kt + 1) * 128]
            if kt < 4:
                nc.tensor.matmul(ps_s[:, 0:CH], ksl,
                                 qf[base:base + 64, 0:CH],
                                 start=True, stop=True)
            nc.tensor.matmul(ps_s[:, CH:T], ksl,
                             qf[base:base + 64, CH:T],
                             start=True, stop=True)
            dcol = kt * 128
            lo = 0 if kt < 4 else CH
            expt = exp_pool.tile([128, T], bf16, tag="expt", name="expt")
            if dcol > lo:
                nc.vector.memset(expt[:, lo:dcol], 0.0)
            nc.scalar.activation(expt[:, dcol:T], ps_s[:, dcol:T], AF.Exp)
            nc.vector.tensor_mul(expt[:, dcol:dcol + 128],
                                 expt[:, dcol:dcol + 128], tri)
            pend_av.append((h_idx, kt, expt))
            if kt == 5:
                while pend_mul:
                    p_y, p_bc, p_base, p_f, p_sl = pend_mul.pop(0)
                    nc.vector.tensor_mul(
                        attn_out[p_base:p_base + 64, p_f, p_sl],
                        p_y[0:64, :], p_bc)
            if len(pend_av) > 2:
                ph, pkt, pex = pend_av.pop(0)
                emit_av(ph, pkt, pex)
                if pkt == NT - 1:
                    finish_head(ph)
        while pend_av:
            ph, pkt, pex = pend_av.pop(0)
            emit_av(ph, pkt, pex)
            if pkt == NT - 1:
                finish_head(ph)
        while pend_mul:
            p_y, p_bc, p_base, p_f, p_sl = pend_mul.pop(0)
            nc.vector.tensor_mul(
                attn_out[p_base:p_base + 64, p_f, p_sl],
                p_y[0:64, :], p_bc)
    return kf_t, qf_t, v1


def _outproj_ln2(nc, tc, h_pool, wc1a_pool, row_pool, x_fm, attn_out,
                 wout_d, wc1_d, bout_c, eps_t, ones_col, ones_row):
    with (
        tc.tile_pool(name="ln2ps", bufs=1, space="PSUM") as stats_ps2,
        tc.tile_pool(name="bc2ps", bufs=1, space="PSUM") as bc_ps2,
        tc.tile_pool(name="ln2_rows", bufs=2) as row_pool2,
        tc.tile_pool(name="ln2_sq", bufs=2) as sq_pool2,
    ):
        rstd2_rows = []
        with (
            tc.tile_pool(name="woutp", bufs=1) as wout_pool,
            tc.tile_pool(name="ops", bufs=3, space="PSUM") as o_ps,
            tc.tile_pool(name="otmp", bufs=2) as otmp_pool,
        ):
            wout_t = []
            for kt in range(F):
                wt = wout_pool.tile([128, C], bf16, tag=f"wout{kt}",
                                    name=f"wout{kt}")
                nc.sync.dma_start(
                    wt, wout_d.ap()[kt * 128:(kt + 1) * 128, :])
                wout_t.append(wt)
            wc1a_t = []
            for kt in range(F):
                wt = wc1a_pool.tile([128, M3 // 2], fp16,
                                    tag=f"wc1a{kt}", name=f"wc1a{kt}")
                nc.sync.dma_start(
                    wt, wc1_d.ap()[kt * 128:(kt + 1) * 128, 0:M3 // 2])
                wc1a_t.append(wt)
            with nc.named_scope("out_proj"):
                for c in range(NCH):
                    sl = _chunk(c)
                    for ct in range(F):
                        ps = o_ps.tile([128, CH], f32, tag="ops",
                                       name="o_ps")
                        for kt in range(F):
                            nc.tensor.matmul(
                                ps, wout_t[kt][:, ct * 128:(ct + 1) * 128],
                                attn_out[:, kt, sl],
                                start=(kt == 0), stop=(kt == F - 1))
                        tmp = otmp_pool.tile([128, CH], f32, tag="otmp",
                                             name="o_tmp")
                        nc.scalar.activation(
                            tmp, ps, AF.Identity, bias=bout_c[:, ct:ct + 1])
                        # x2 = x + attn_proj, in place in the residual slot
                        nc.vector.tensor_add(
                            x_fm[:, ct, sl], tmp, x_fm[:, ct, sl])
            with nc.named_scope("ln2"):
                for c in range(NCH):
                    sl = _chunk(c)
                    ps_sum = stats_ps2.tile([1, CH], f32, tag="lnsum",
                                            name=f"l2sum{c}")
                    ps_sq = stats_ps2.tile([1, CH], f32, tag="lnsq",
                                           name=f"l2sq{c}")
                    for f in range(F):
                        sq_t = sq_pool2.tile([128, CH], bf16, tag="ln_sq",
                                             name="sq_t")
                        nc.scalar.activation(sq_t, x_fm[:, f, sl],
                                             AF.Square)
                        nc.tensor.matmul(ps_sum, ones_col, x_fm[:, f, sl],
                                         start=(f == 0), stop=(f == F - 1))
                        nc.tensor.matmul(ps_sq, ones_col, sq_t,
                                         start=(f == 0), stop=(f == F - 1))
                    mean = row_pool2.tile([1, CH], f32, tag="l2ra",
                                          name="mean")
                    nc.vector.tensor_scalar_mul(mean, ps_sum, 1.0 / C)
                    m2 = row_pool2.tile([1, CH], f32, tag="l2rb", name="m2")
                    nc.vector.tensor_scalar_mul(m2, ps_sq, 1.0 / C)
                    msq = row_pool2.tile([1, CH], f32, tag="l2rc",
                                         name="msq")
                    nc.vector.tensor_mul(msq, mean, mean)
                    nc.vector.tensor_sub(m2, m2, msq)   # m2 <- var
                    std2 = row_pool2.tile([1, CH], f32, tag="l2rd",
                                          name="std2")
                    nc.scalar.activation(std2, m2, AF.Sqrt, bias=eps_t)
                    stdc = row_pool2.tile([1, CH], f32, tag="l2rf",
                                          name="stdc2")
                    nc.vector.tensor_copy(stdc, std2)
                    r0 = row_pool2.tile([1, CH], f32, tag="l2re",
                                        name="r0")
                    nc.vector.reciprocal_approx_fast(out=r0, in_=stdc)
                    rrow = row_pool.tile([1, CH], f32r, tag=f"rstd2_{c}",
                                         name=f"rstd2_{c}")
                    nc.vector.tensor_copy(rrow, r0)
                    rstd2_rows.append(rrow)

        h2_fm = h_pool.tile([128, F, T], fp16, tag="h_slot", name="h2_fm")
        with nc.named_scope("ln2h"):
            for c in range(NCH):
                sl = _chunk(c)
                ps_bc = bc_ps2.tile([128, CH], f32, tag="bc2",
                                    name="ps_bc2")
                nc.tensor.matmul(
                    ps_bc, ones_row, rstd2_rows[c],
                    start=True, stop=True)
                for f in range(F):
                    nc.vector.tensor_mul(
                        h2_fm[:, f, sl], x_fm[:, f, sl], ps_bc)
    return h2_fm, wc1a_t


def _mlp(nc, tc, wc1a_t, h2_fm, x2_fm, out_fm, wc1_d, wc2_d, bc1_c, bc2_c,
         ident, y_d):
    with (
        tc.tile_pool(name="wc1b", bufs=1) as wc1b_pool,
        tc.tile_pool(name="wc2s", bufs=3) as wc2_pool,
        tc.tile_pool(name="mlpout", bufs=1, space="PSUM") as mo_ps,
        tc.tile_pool(name="mlpc1", bufs=2, space="PSUM") as c1_ps,
        tc.tile_pool(name="gp", bufs=3) as g_pool,
        tc.tile_pool(name="mtmp", bufs=2) as mtmp_pool,
        tc.tile_pool(name="otm", bufs=2) as otm_pool,
    ):
        wc1b_t = []
        for kt in range(F):
            wt = wc1b_pool.tile([128, M3 // 2], fp16, tag=f"wc1b{kt}",
                                name=f"wc1b{kt}")
            nc.sync.dma_start(
                wt, wc1_d.ap()[kt * 128:(kt + 1) * 128, M3 // 2:M3])
            wc1b_t.append(wt)
        for half in range(2):
            wc1_t = wc1a_t if half == 0 else wc1b_t
            with nc.named_scope(f"mlp_h{half}"):
                for c in range(NCH):
                    sl = _chunk(c)
                    wc2_t = []
                    for mt in range(MTH):
                        row0 = (half * MTH + mt) * 128
                        wt = wc2_pool.tile([128, C], fp16, tag="wc2",
                                           name=f"wc2_{mt}")
                        nc.sync.dma_start(
                            wt, wc2_d.ap()[row0:row0 + 128, :])
                        wc2_t.append(wt)
                    ps_out = [mo_ps.tile([128, CH], f32, tag=f"mo{ct}",
                                         name=f"mo{ct}")
                              for ct in range(F)]
                    g_prev = None
                    for mt in range(MTH):
                        ps_g = c1_ps.tile([128, CH], f32, tag="c1ps",
                                          name="ps_g")
                        for kt in range(F):
                            nc.tensor.matmul(
                                ps_g,
                                wc1_t[kt][:, mt * 128:(mt + 1) * 128],
                                h2_fm[:, kt, sl],
                                start=(kt == 0), stop=(kt == F - 1))
                        if mt > 0:
                            for ct in range(F):
                                nc.tensor.matmul(
                                    ps_out[ct],
                                    wc2_t[mt - 1]
                                    [:, ct * 128:(ct + 1) * 128],
                                    g_prev,
                                    start=(mt == 1), stop=False)
                        g_t = g_pool.tile([128, CH], fp16, tag="g",
                                          name="g_t")
                        nc.scalar.activation(
                            g_t, ps_g, AF.Gelu,
                            bias=bc1_c[:, half * MTH + mt:
                                       half * MTH + mt + 1])
                        g_prev = g_t
                    for ct in range(F):
                        nc.tensor.matmul(
                            ps_out[ct],
                            wc2_t[MTH - 1][:, ct * 128:(ct + 1) * 128],
                            g_prev, start=False, stop=True)
                    for ct in range(F):
                        if half == 0:
                            nc.vector.tensor_add(
                                out_fm[:, ct, sl], ps_out[ct],
                                x2_fm[:, ct, sl])
                        else:
                            tmp = mtmp_pool.tile([128, CH], f32,
                                                 tag="mtmp", name="m_tmp")
                            nc.scalar.activation(
                                tmp, ps_out[ct], AF.Identity,
                                bias=bc2_c[:, ct:ct + 1])
                            nc.vector.tensor_add(
                                out_fm[:, ct, sl], out_fm[:, ct, sl], tmp)
                    if half == 1:
                        with nc.named_scope(f"store_c{c}"):
                            for t in range(4 * c, 4 * (c + 1)):
                                o_tm = otm_pool.tile([128, C], f32,
                                                     tag="o_tm",
                                                     name="o_tm")
                                for f in range(F):
                                    ps = mo_ps.tile([128, 128], f32,
                                                    tag=f"mo{f}",
                                                    name="otr")
                                    nc.tensor.transpose(
                                        ps,
                                        out_fm[:, f,
                                               t * 128:(t + 1) * 128],
                                        ident)
                                    nc.vector.tensor_copy(
                                        o_tm[:, f * 128:(f + 1) * 128],
                                        ps)
                                nc.sync.dma_start(
                                    y_d.ap()[t * 128:(t + 1) * 128, :],
                                    o_tm)


def _get_nc():
    global _NC_CACHE
    if _NC_CACHE is None:
        _NC_CACHE = _build()
    return _NC_CACHE


def _prep_weights(inputs):
    """Host-side algebraic folding; returns the device weight dict."""
    f32c = lambda a: np.ascontiguousarray(np.asarray(a, np.float64))
    ln_w = f32c(inputs["ln_w"])
    ln_b = f32c(inputs["ln_b"])
    w_qkv = f32c(inputs["w_qkv"])
    b_qkv = f32c(inputs["b_qkv"])
    w_out = f32c(inputs["w_out"])
    b_out = f32c(inputs["b_out"])
    w_c1 = f32c(inputs["w_c1"])
    b_c1 = f32c(inputs["b_c1"])
    w_c2 = f32c(inputs["w_c2"])
    b_c2 = f32c(inputs["b_c2"])

    wqkv_lw = ln_w[:, None] * w_qkv
    wqkv_eff = wqkv_lw - wqkv_lw.mean(axis=0, keepdims=True)
    bqkv_eff = ln_b @ w_qkv + b_qkv
    bq_eff = bqkv_eff[C:2 * C]                  # k bias cancels in softmax
    bv_eff = bqkv_eff[2 * C:3 * C]
    bout_eff = b_out + bv_eff @ w_out           # v bias folds through Wout
    wc1_lw = ln_w[:, None] * w_c1
    wc1_eff = wc1_lw - wc1_lw.mean(axis=0, keepdims=True)
    bc1_eff = ln_b @ w_c1 + b_c1

    c16 = lambda a: np.ascontiguousarray(a.astype(np.float16))
    cbf = lambda a: np.ascontiguousarray(a.astype(ml_dtypes.bfloat16))
    cf = lambda a: np.ascontiguousarray(a.astype(np.float32))
    return {
        "wqkv": c16(wqkv_eff), "bq": cf(bq_eff),
        "wout": cbf(w_out), "bout": cf(bout_eff),
        "wc1": c16(wc1_eff), "bc1": cf(bc1_eff),
        "wc2": c16(w_c2), "bc2": cf(b_c2),
    }


def run(inputs, trace=False):
    nc = _get_nc()
    xs = np.asarray(inputs["x"], dtype=np.float32)
    assert xs.shape == (B, T, C), xs.shape
    xs = np.ascontiguousarray(xs.astype(ml_dtypes.bfloat16))
    shared = _prep_weights(inputs)
    in_maps = [dict(shared, x=xs[c]) for c in range(B)]
    res = bass_utils.run_bass_kernel_spmd(
        nc, in_maps, core_ids=list(range(B)), trace=trace)
    out = np.stack([r["y"] for r in res.results], axis=0)
    return out, res


def kernel(**inputs):
    out, _ = run(inputs, trace=False)
    return out
